# revision 39
# baseline (speedup 1.0000x reference)
"""Batch-all triplet loss on 8 TRN2 NeuronCores.

Strategy (data-parallel over anchors, per sharding hint):
- Host sorts rows by class so each class is a contiguous block; each core
  owns 64 anchor rows of the sorted order.
- Each core receives the full feature matrix, transposed and column-reordered
  so its 128-column "band" (covering every anchor's class window) comes first,
  in bf16.  Two extra contraction rows fold the per-column squared norms into
  the Gram matmul, so PSUM directly accumulates  dot(i,k) - sq_k/2  and a
  single scaled copy (x -2) yields  Dt[i,k] = sq_k - 2 dot(i,k)
  (= dist(i,k) - sq_i; the sq_i term cancels in every hinge difference).
- The per-anchor class window (the positives' distances) is pulled out of the
  band with one indirect DMA gather; those values (+margin) become per-
  partition biases.
- Main hinge term: for window offset j, sum_k relu(Dt[i,p_j] + m - Dt[i,k])
  over ALL k via one fused instruction per j (DVE scalar_tensor_tensor with
  min+accum, or ACT Relu with bias+accum), with two offsets stacked in the
  128-partition dim.  The same-class part of that k-sum (the "correction")
  plus the tiny denominator bookkeeping is reproduced exactly on the host
  from the gathered window values.
"""

import os
import numpy as np
import ml_dtypes

N = 512
DDIM = 2048
NCORE = 8
RPC = N // NCORE          # 64 anchor rows per core
MAXM = 32                 # max class size supported (window width)
BAND = 128                # band columns (window always inside)
NCOL = N + MAXM           # 544 columns incl. pads
KCH = DDIM // 128         # 16 contraction chunks
MARGIN = 200.0
SW = 64                   # gathered superwindow width (32-aligned rows x2)
TD = 26                   # main-loop iterations on DVE
TA = 6                    # main-loop iterations on ACT  (TD+TA == SW/2)
NWARM = 8                 # PE warm-up matmuls

_prog_cache = {}


def build_program():
    """Build the SPMD Bass program (same program for all 8 cores)."""
    if "nc" in _prog_cache:
        return _prog_cache["nc"]
    import concourse.bass as bass
    import concourse.bacc as bacc
    import concourse.mybir as mybir
    import concourse.tile as tile
    from concourse.tile import add_dep_helper

    dt = mybir.dt
    nc = bacc.Bacc("TRN2", target_bir_lowering=False, debug=False)

    xt_d = nc.dram_tensor("xt", [DDIM, NCOL], dt.bfloat16, kind="ExternalInput").ap()
    aug_d = nc.dram_tensor("aug", [2, NCOL], dt.bfloat16, kind="ExternalInput").ap()
    offs_d = nc.dram_tensor("offs", [128, 2], dt.int32, kind="ExternalInput").ap()
    accd_d = nc.dram_tensor("acc_dve", [128, TD], dt.float32, kind="ExternalOutput").ap()
    acca_d = nc.dram_tensor("acc_act", [128, TA], dt.float32, kind="ExternalOutput").ap()
    wout_d = nc.dram_tensor("wout", [128, SW], dt.float32, kind="ExternalOutput").ap()
    band_d = nc.dram_tensor("band", [RPC * BAND], dt.float32, kind="Internal").ap()

    with tile.TileContext(nc) as tc:
        with (
            tc.tile_pool(name="big", bufs=1) as big,
            tc.tile_pool(name="small", bufs=1) as small,
            tc.tile_pool(name="scr", bufs=6) as scr,
            tc.tile_pool(name="psum", bufs=1, space="PSUM") as ppool,
        ):
            xt_sb = big.tile([128, KCH * NCOL], dt.bfloat16)
            d2 = big.tile([128, NCOL], dt.float16)
            dummy = big.tile([128, 512], dt.bfloat16)
            aug_sb = small.tile([2, NCOL], dt.bfloat16)
            ones2 = small.tile([2, RPC], dt.bfloat16)
            offs_sb = small.tile([128, 2], dt.int32)
            offs_pl = small.tile([128, 2], dt.int32)
            wg = small.tile([128, SW], dt.float32)
            band_sb = small.tile([RPC, BAND], dt.float32)
            accd_sb = small.tile([128, TD], dt.float32)
            acca_sb = small.tile([128, TA], dt.float32)

            pa = ppool.tile([RPC, BAND], dt.float32)
            pb = ppool.tile([RPC, NCOL - BAND], dt.float32)
            pdum = ppool.tile([128, 512], dt.float32)

            # big input DMAs first: two halves of the K dimension
            half = (KCH // 2) * NCOL
            nc.sync.dma_start(
                out=xt_sb[:, 0:half].rearrange("p (c m) -> p c m", m=NCOL),
                in_=xt_d[0 : DDIM // 2, :].rearrange("(c p) m -> p c m", p=128),
            )
            nc.sync.dma_start(
                out=xt_sb[:, half : 2 * half].rearrange("p (c m) -> p c m", m=NCOL),
                in_=xt_d[DDIM // 2 : DDIM, :].rearrange("(c p) m -> p c m", p=128),
            )

            # constants / tiny inputs
            nc.vector.memset(dummy[:, :], 0.0)
            nc.vector.memset(ones2[:, :], 1.0)
            nc.sync.dma_start(out=aug_sb[:, :], in_=aug_d[:, :])
            nc.sync.dma_start(out=offs_sb[:, :], in_=offs_d[:, :])

            # PE warm-up (HAM ramp) on a scratch PSUM bank
            for _ in range(NWARM):
                nc.tensor.matmul(
                    pdum[:, :], lhsT=dummy[:, 0:128], rhs=dummy[:, :],
                    start=True, stop=True,
                )

            # Gram matmuls.  lhsT = this core's 64 anchor columns (band
            # positions 32..96); group A = band columns, group B = the rest.
            def mm(group_out, col_lo, col_hi, c, start):
                nc.tensor.matmul(
                    group_out,
                    lhsT=xt_sb[:, c * NCOL + 32 : c * NCOL + 96],
                    rhs=xt_sb[:, c * NCOL + col_lo : c * NCOL + col_hi],
                    start=start, stop=False,
                )

            for c in range(KCH // 2):
                mm(pa[:, :], 0, BAND, c, c == 0)
            for c in range(KCH // 2):
                mm(pb[:, :], BAND, NCOL, c, c == 0)
            for c in range(KCH // 2, KCH):
                mm(pa[:, :], 0, BAND, c, False)
            # augmented rows fold +sq_k/2 (negated) into the accumulation
            nc.tensor.matmul(
                pa[:, :], lhsT=ones2[:, :], rhs=aug_sb[:, 0:BAND],
                start=False, stop=True,
            )
            for c in range(KCH // 2, KCH):
                mm(pb[:, :], BAND, NCOL, c, False)
            nc.tensor.matmul(
                pb[:, :], lhsT=ones2[:, :], rhs=aug_sb[:, BAND:NCOL],
                start=False, stop=True,
            )

            # fp32 band (Dt_shifted, no margin) -> DRAM -> indirect gather of
            # each anchor's class superwindow; gathered values = fp32 biases.
            # Emitted FIRST so ACT serves the gather chain before anything else.
            ActF = mybir.ActivationFunctionType
            Alu = mybir.AluOpType
            nc.scalar.activation(
                out=band_sb[:, :], in_=pa[:, :], func=ActF.Copy, scale=-2.0,
            )
            band_dma = nc.sync.dma_start(
                out=band_d.rearrange("(p m) -> p m", p=RPC)[:, :],
                in_=band_sb[:, :],
            )

            # PSUM -> SBUF:  d2 = Dt_shifted - margin  (fp16, margin folded
            # in so the gathered fp32 band values serve as biases).  Band part
            # on ACT and rest part on DVE run in parallel; each region's
            # partition-duplication DMA fires as soon as its copy lands.
            nc.scalar.activation(
                out=d2[0:RPC, 0:BAND], in_=pa[:, :], func=ActF.Copy,
                scale=-2.0, bias=-MARGIN,
            )
            nc.vector.tensor_scalar(
                out=d2[0:RPC, BAND:NCOL], in0=pb[:, :], scalar1=-2.0,
                scalar2=-MARGIN, op0=Alu.mult, op1=Alu.add,
            )
            nc.gpsimd.dma_start(out=d2[RPC:128, :], in_=d2[0:RPC, :])
            # 32-aligned row view of the band; gather the two aligned 32-wide
            # rows covering each anchor's class window (HW indirect DMA is
            # row-granular: verified exact for row-aligned sources).
            band_rows = band_d.rearrange("(r m) -> r m", m=32)
            # stage offsets through a DVE op that also (artificially) depends
            # on the band DMA: each gather then needs only ONE semaphore wait
            # (walrus limit for DMA instructions).
            cp = nc.vector.tensor_scalar(
                out=offs_pl[:, :], in0=offs_sb[:, :], scalar1=0,
                scalar2=None, op0=mybir.AluOpType.add,
            )
            add_dep_helper(cp.ins, band_dma.ins, sync=True, reason="gather join")
            nc.gpsimd.indirect_dma_start(
                out=wg[:, 0:32], out_offset=None, in_=band_rows,
                in_offset=bass.IndirectOffsetOnAxis(ap=offs_pl[:, 0:1], axis=0),
            )
            nc.gpsimd.indirect_dma_start(
                out=wg[:, 32:SW], out_offset=None, in_=band_rows,
                in_offset=bass.IndirectOffsetOnAxis(ap=offs_pl[:, 1:2], axis=0),
            )
            nc.sync.dma_start(out=wout_d[:, :], in_=wg[:, :])

            # main hinge loop: per window offset, fused (bias - Dt) relu + row-sum
            # DVE: acc = sum_k min(Dt, b)  (fp16 2x mode); host converts via
            # sum_k relu(b - Dt) = NCOL*b - acc.  ACT: direct relu+accum.
            for t in range(TD):
                s = scr.tile([128, NCOL], dt.float16, tag="sd")
                nc.vector.tensor_scalar(
                    out=s[:, :],
                    in0=d2[:, :],
                    scalar1=wg[:, t : t + 1],
                    scalar2=0.0,
                    op0=Alu.min,
                    op1=Alu.add,
                    accum_out=accd_sb[:, t : t + 1],
                )
            for t in range(TA):
                s = scr.tile([128, NCOL], dt.float32, tag="sa")
                nc.scalar.activation(
                    out=s[:, :],
                    in_=d2[:, :],
                    func=ActF.Relu,
                    bias=wg[:, TD + t : TD + t + 1],
                    scale=-1.0,
                    accum_out=acca_sb[:, t : t + 1],
                )

            nc.sync.dma_start(out=accd_d[:, :], in_=accd_sb[:, :])
            nc.sync.dma_start(out=acca_d[:, :], in_=acca_sb[:, :])

    nc.compile()
    _prog_cache["nc"] = nc
    return nc


def prep_host(inputs_np, targets_np):
    """All host-side preprocessing derived from inputs/targets."""
    X = np.asarray(inputs_np, dtype=np.float32)
    T = np.asarray(targets_np).astype(np.int64)
    assert X.shape == (N, DDIM) and T.shape == (N,)

    order = np.argsort(T, kind="stable")
    Xs = X[order]
    Ts = T[order]
    Xb = Xs.astype(ml_dtypes.bfloat16)           # device sees these bits
    Xb32 = Xb.astype(np.float32)
    sq = np.sum(Xb32 * Xb32, axis=1, dtype=np.float32)   # [N] fp32

    # class block start / size per sorted row
    classes, starts, counts = np.unique(Ts, return_index=True, return_counts=True)
    assert counts.max() <= MAXM, f"class size {counts.max()} > MAXM"
    bs = np.zeros(N, np.int64)
    ms = np.zeros(N, np.int64)
    for s0, cnt in zip(starts, counts):
        bs[s0 : s0 + cnt] = s0
        ms[s0 : s0 + cnt] = cnt

    per_core = []
    for c in range(NCORE):
        r0 = c * RPC
        b0 = r0 - 32
        band_cols = np.arange(b0, b0 + BAND)
        okb = (band_cols >= 0) & (band_cols < N)
        rest = np.setdiff1d(np.arange(N), band_cols[okb])
        col_ids = np.concatenate([band_cols, rest, -np.ones(NCOL - BAND - len(rest), np.int64)])
        ok = (col_ids >= 0) & (col_ids < N)
        cid = np.clip(col_ids, 0, N - 1)

        xt = np.where(ok[None, :], Xb32[cid].T, np.float32(0.0)).astype(ml_dtypes.bfloat16)
        # pad sentinel: Dt_shifted = sq - 2048 = 60000, finite in fp16
        sqc = np.where(ok, sq[cid], np.float32(62048.0)).astype(np.float32)
        # psum accumulates dot - sq/2 + 1024, so d2 = -2*psum = sq - 2dot - 2048
        t_half = (np.float32(1024.0) - sqc / np.float32(2.0)).astype(np.float32)
        hi = t_half.astype(ml_dtypes.bfloat16)
        lo = (t_half - hi.astype(np.float32)).astype(ml_dtypes.bfloat16)
        aug = np.stack([hi, lo])                                  # [2, NCOL]

        rows = np.arange(r0, r0 + RPC)
        offs_row = (bs[rows] - b0).astype(np.int64)               # window start in band
        assert offs_row.min() >= 0 and (offs_row + MAXM).max() <= BAND
        ra = (np.arange(RPC) * BAND + offs_row) // 32             # aligned 32-row id
        assert (ra + 1).max() <= RPC * BAND // 32 - 1
        # gather col-block 0 = this partition's bias half: rows p<64 take the
        # first aligned row (superwindow cols 0..31), rows p>=64 the second.
        o_lo = np.stack([ra, ra + 1], axis=1).astype(np.int32)    # [64, 2]
        o_hi = np.stack([ra + 1, ra], axis=1).astype(np.int32)
        offs = np.concatenate([o_lo, o_hi], axis=0)               # [128, 2]

        sw0 = (offs_row // 32) * 32                               # superwindow start (band coords)
        jg = np.arange(SW)[None, :]
        gcol = b0 + sw0[:, None] + jg                             # global sorted col id
        inblk = (gcol >= bs[rows][:, None]) & (gcol < (bs[rows] + ms[rows])[:, None])
        validP = inblk & (gcol != rows[:, None])
        validK = inblk

        per_core.append(
            dict(xt=np.ascontiguousarray(xt), aug=aug, offs=offs,
                 validP=validP, validK=validK)
        )

    # --- denominator bookkeeping (host, matches the jax reference) ---
    try:
        import jax
        import jax.numpy as jnp

        cpu = jax.devices("cpu")[0]
        with jax.default_device(cpu):
            jX = jnp.asarray(X)
            dd = jnp.sum(jX * jX, axis=1) * 2.0 - 2.0 * jnp.diagonal(jnp.matmul(jX, jX.T))
            n_self_valid = int(jnp.sum(dd > 1e-9))
    except Exception:
        dots = X @ X.T
        s2 = np.sum(X * X, axis=1)
        n_self_valid = int(np.sum(s2 * 2 - 2 * np.diagonal(dots) > 1e-9))

    count = int(np.sum(counts * (counts - 1))) + n_self_valid
    # last anchor (original order) with a valid positive; class sizes >= 2
    # make every anchor valid, so this is simply the last row.
    m_last = int(counts[np.searchsorted(classes, T[N - 1])])
    neg_pairs = N - m_last
    denom = np.float32(count) * np.float32(neg_pairs)

    return per_core, denom


def combine_host(per_core, results, denom):
    """Reduce per-core device outputs to the final scalar (fp64 on host)."""
    main_total = 0.0
    corr_total = 0.0
    for c in range(NCORE):
        pc = per_core[c]
        res = results[c]
        accd = np.asarray(res["acc_dve"], dtype=np.float32)   # [128, TD]
        acca = np.asarray(res["acc_act"], dtype=np.float32)   # [128, TA]
        # wout: fp32 Dt_shifted (= the device bias values); rows 0..63 hold
        # [w(0:32), w(32:64)] per anchor
        w32 = np.asarray(res["wout"], dtype=np.float32)[0:RPC]    # [64, SW]
        validP = pc["validP"]
        validK = pc["validK"]

        # device bias = gathered fp32 value; device d2 = fp16(bias - margin)
        bias = w32
        bias64 = bias.astype(np.float64)
        d16 = np.float16(w32 - np.float32(MARGIN)).astype(np.float32)   # [64, SW]
        # device main sums: partition p handles anchor p%64, superwindow col
        # j = 32*(p//64) + t.  DVE columns hold sum_k min(Dt, b) -> main =
        # NCOL*b - acc;  ACT columns hold main directly.
        main = np.zeros((RPC, SW), np.float64)
        is_dve = np.zeros(SW, bool)
        for half in range(2):
            rowsl = slice(half * RPC, (half + 1) * RPC)
            js = slice(half * 32, half * 32 + TD)
            is_dve[js] = True
            main[:, js] = NCOL * bias64[:, js] - accd[rowsl, :].astype(np.float64)
            main[:, half * 32 + TD : half * 32 + 32] = acca[rowsl, :].astype(np.float64)
        main_total += float(np.sum(main * validP))

        # correction: same-class k part, replicating each path's arithmetic.
        # DVE cols (acc = sum fp16(min(Dt,b))): block part of main-estimate is
        # b - fp16(min(w',b)).  ACT cols: fp32 relu(b - w').
        pairs = validP[:, :, None] & validK[:, None, :]               # [64, SW, SW]
        mind = np.float16(np.minimum(d16[:, None, :], bias[:, :, None])).astype(np.float64)
        corr_dve = bias64[:, :, None] - mind
        corr_act = np.maximum(bias64[:, :, None] - d16.astype(np.float64)[:, None, :], 0.0)
        corr = np.where(is_dve[None, :, None], corr_dve, corr_act)
        corr_total += float(np.sum(corr * pairs))

    loss_sum = main_total - corr_total
    return np.asarray(np.float32(np.float32(loss_sum) / denom))


def kernel(**inputs):
    from concourse import bass_utils

    per_core, denom = prep_host(inputs["inputs"], inputs["targets"])
    nc = build_program()
    in_maps = [
        {"xt": pc["xt"], "aug": pc["aug"], "offs": pc["offs"]} for pc in per_core
    ]
    out = bass_utils.run_bass_kernel_spmd(nc, in_maps, core_ids=list(range(NCORE)))
    return combine_host(per_core, out.results, denom)
